# revision 3
# baseline (speedup 1.0000x reference)
"""Bass/Tile TRN2 kernel for nn_BatchGraphAttentionLayer (v3).

Reference computation (per batch b):
    Wh  = h[b] @ W                    # [64, 256]
    s1  = Wh @ a[:256], s2 = Wh @ a[256:]
    e   = leaky_relu(s1[i] + s2[j])   # [64, 64] (i rows, j cols)
    att = softmax over axis i of where(adj[i,j]>0, e, -9e15)
    out = elu(att @ Wh)               # h_prime[i] = sum_j att[i,j] Wh[j]

Sharding: data-parallel over batch, 8 cores x 4 batches.

v3 design (from v2, after trace analysis: the kernel is HBM-stream
bound at ~350 GB/s for 16.7 MB/core, with a ~7 us preamble and a
~9 us post-stream tail; the PE ran at 1.2 GHz for the first ~28 us
because HAM saw only intermittent work early):
  - a folded into W on host: w_ext = [W | W@a1 | W@a2] bf16 [16384,258].
  - h and w_ext repacked on host so every DMA slab is one fully
    contiguous per-partition run (hpack/wpack [128, cols]); h streams
    on the sync HWDGE ring, w on the scalar one.  All slab DMAs are
    issued up front into dedicated SBUF buffers (no reuse stalls).
  - ~20 dummy rank-1 matmuls at program start keep the PE busy during
    the DMA-latency window so HAM un-throttles to 2.4 GHz before the
    real stream arrives.
  - attention: batches pair-stacked 2 per 128-partition tile; the
    logit tile for BOTH pairs is built in one PSUM tile [128, 256]:
      e2[j, i+128t] = s1^t[i] + maskpre[j, i]
    via two accumulating matmuls (ones x s1rows, masknegT x [I|I]).
    maskpre = 0 where adj allows, -250 where masked: after the
    leaky-relu (alpha=0.2) that becomes <= -45, so exp() kills the
    entry (rel contribution < 1e-19) -- replaces the explicit 0/1
    mask multiply.
  - softmax: one ACT Prelu (bias = per-partition s2), one ACT Exp
    with accum_out giving the row sums in the same pass, reciprocal +
    scale on DVE, att in bf16.
  - out = elu(att @ Wh) with att/Wh bf16; elu = min(exp(x),1)-1+max(x,0).
"""

import os
from contextlib import ExitStack

import ml_dtypes
import numpy as np

import concourse.bass as bass
import concourse.tile as tile
from concourse import bacc, mybir
from concourse.bass_utils import run_bass_kernel_spmd

F32 = mybir.dt.float32
BF16 = mybir.dt.bfloat16

B, N, IN, OUT = 32, 64, 16384, 256
OUTE = OUT + 2               # w_ext columns (W | Wa1 | Wa2)
NCORES = 8
BPC = B // NCORES            # batches per core = 4
M = BPC * N                  # local rows = 256
P = 128
ALPHA = 0.2
MASKPRE = -250.0             # pre-leaky additive mask (-50 post-leaky)

KSUB = IN // P               # 128 k-subtiles of 128 rows
# taper both ends: fast PE ramp-up, short post-stream crunch
SLABS = [1, 1, 2, 4, 8, 16, 16, 16, 16, 16, 16, 8, 4, 2, 1, 1]
assert sum(SLABS) == KSUB
NWARM = int(os.environ.get("GAT_NWARM", "20"))

_NC = None
LAST_EXEC_NS = None
LAST_RESULTS = None


def _build_kernel(ctx: ExitStack, tc: tile.TileContext, out, hp, wp,
                  eye, masknegT, eyeext):
    nc = tc.nc

    consts = ctx.enter_context(tc.tile_pool(name="consts", bufs=1))
    hpool = ctx.enter_context(tc.tile_pool(name="hslab", bufs=1))
    wpool = ctx.enter_context(tc.tile_pool(name="wslab", bufs=1))
    whpool = ctx.enter_context(tc.tile_pool(name="wh", bufs=1))
    attp = ctx.enter_context(tc.tile_pool(name="att", bufs=1))
    ps_accp = ctx.enter_context(tc.tile_pool(name="psacc", bufs=1, space="PSUM"))
    ps_smallp = ctx.enter_context(tc.tile_pool(name="pssmall", bufs=1, space="PSUM"))
    ps_ep = ctx.enter_context(tc.tile_pool(name="pse", bufs=1, space="PSUM"))
    ps_op = ctx.enter_context(tc.tile_pool(name="pso", bufs=1, space="PSUM"))
    ps_warmp = ctx.enter_context(tc.tile_pool(name="pswarm", bufs=1, space="PSUM"))

    # ---- constants (gpsimd/SWDGE keeps the HWDGE queues clear) ----
    sb_eye = consts.tile([P, P], F32)
    nc.gpsimd.dma_start(sb_eye, eye)
    sb_mneg = consts.tile([P, P], BF16)
    nc.gpsimd.dma_start(sb_mneg, masknegT)
    sb_eyex = consts.tile([P, 2 * P], BF16)
    nc.gpsimd.dma_start(sb_eyex, eyeext)
    # ones row for the rank-1 e-matmul (partition 0 only: legal base)
    sb_ones1 = consts.tile([1, P], BF16)
    nc.vector.memset(sb_ones1, 1.0)
    sb_warm = consts.tile([1, 2 * P], BF16)
    nc.vector.memset(sb_warm, 0.0)

    # ---- stream DMAs: all issued up front, dedicated buffers ----
    hs = []
    ws = []
    k0 = 0
    for s, nsub in enumerate(SLABS):
        ht = hpool.tile([P, nsub * M], BF16, tag=f"h{s}", name=f"h{s}")
        nc.sync.dma_start(ht, hp[:, k0 * M:(k0 + nsub) * M])
        wt = wpool.tile([P, nsub * OUTE], BF16, tag=f"w{s}", name=f"w{s}")
        nc.scalar.dma_start(wt, wp[:, k0 * OUTE:(k0 + nsub) * OUTE])
        hs.append(ht)
        ws.append(wt)
        k0 += nsub

    # ---- HAM warm-up: keep the PE busy while the first slabs land ----
    ps_warm = ps_warmp.tile([P, 2 * P], F32, tag="ps_warm", name="ps_warm")
    for i in range(NWARM):
        nc.tensor.matmul(ps_warm, lhsT=sb_warm[:, 0:P], rhs=sb_warm,
                         start=True, stop=True, skip_group_check=True)

    # ---- projection: Wh_ext accumulated in PSUM (m-major) ----
    ps_wh = [ps_accp.tile([P, OUTE], F32, tag=f"ps_wh{t}", name=f"ps_wh{t}")
             for t in range(2)]
    nslab = len(SLABS)
    k0 = 0
    for s, nsub in enumerate(SLABS):
        first = (s == 0)
        last = (s == nslab - 1)
        for c in range(nsub):
            for t in range(2):
                nc.tensor.matmul(ps_wh[t],
                                 lhsT=hs[s][:, c * M + t * P:c * M + (t + 1) * P],
                                 rhs=ws[s][:, c * OUTE:(c + 1) * OUTE],
                                 start=(first and c == 0),
                                 stop=(last and c == nsub - 1),
                                 skip_group_check=True)
        k0 += nsub

    # ---- logits: e2[j, i+128t] = maskpre[j,i] + s1^t[i] ----
    # mask matmul first: depends only on consts, so the PE can run it
    # in any stream-starvation gap (it sits right after the projection
    # in the FIFO).
    ps_e2 = ps_ep.tile([P, 2 * P], F32, tag="ps_e2", name="ps_e2")
    nc.tensor.matmul(ps_e2, lhsT=sb_mneg, rhs=sb_eyex,
                     start=True, stop=False, skip_group_check=True)

    # scores: s1/s2 columns -> one PE transpose; s2 stays a column and
    # becomes the Prelu bias
    sc = attp.tile([P, 4], F32, tag="sc")
    nc.vector.tensor_copy(out=sc[:, 0:2], in_=ps_wh[0][:, OUT:OUTE])
    nc.scalar.copy(out=sc[:, 2:4], in_=ps_wh[1][:, OUT:OUTE])
    pst0 = ps_smallp.tile([1, P], F32, tag="pst0", name="pst0")
    pst1 = ps_smallp.tile([1, P], F32, tag="pst1", name="pst1")
    nc.tensor.transpose(pst0, sc[:, 0:1], sb_eye)
    nc.tensor.transpose(pst1, sc[:, 2:3], sb_eye)
    s1row = attp.tile([1, 2 * P], BF16, tag="s1row")
    nc.vector.tensor_copy(out=s1row[:, 0:P], in_=pst0[0:1, :])
    nc.scalar.copy(out=s1row[:, P:2 * P], in_=pst1[0:1, :])
    nc.tensor.matmul(ps_e2, lhsT=sb_ones1, rhs=s1row,
                     start=False, stop=True, skip_group_check=True)

    # Wh to SBUF in bf16 (moving operand of the output matmul)
    wh_m = [whpool.tile([P, OUT], BF16, tag=f"wh_m{t}", name=f"wh_m{t}")
            for t in range(2)]
    nc.vector.tensor_copy(out=wh_m[0], in_=ps_wh[0][:, :OUT])
    nc.vector.tensor_copy(out=wh_m[1], in_=ps_wh[1][:, :OUT])

    # ---- softmax: Prelu(+s2), Exp with fused row-sum, scale ----
    rs = attp.tile([P, 2], F32, tag="rs")
    rinv = attp.tile([P, 2], F32, tag="rinv")
    att = attp.tile([P, 2 * P], BF16, tag="attw")
    for t in range(2):
        lk = attp.tile([P, P], F32, tag=f"lk{t}", name=f"lk{t}")
        nc.scalar.activation(lk, ps_e2[:, t * P:(t + 1) * P],
                             mybir.ActivationFunctionType.Prelu,
                             bias=sc[:, 2 * t + 1:2 * t + 2], alpha=ALPHA)
        pexp = attp.tile([P, P], F32, tag=f"pexp{t}", name=f"pexp{t}")
        nc.scalar.activation(pexp, lk, mybir.ActivationFunctionType.Exp,
                             accum_out=rs[:, t:t + 1])
        nc.vector.reciprocal(rinv[:, t:t + 1], rs[:, t:t + 1])
        nc.vector.tensor_scalar_mul(att[:, t * P:(t + 1) * P], pexp,
                                    rinv[:, t:t + 1])

    # ---- out = elu(att @ Wh), pair-interleaved ----
    for t in range(2):
        ps_o = ps_op.tile([P, OUT], F32, tag=f"ps_o{t}", name=f"ps_o{t}")
        nc.tensor.matmul(ps_o, lhsT=att[:, t * P:(t + 1) * P], rhs=wh_m[t],
                         start=True, stop=True, skip_group_check=True)
        # elu(x) = min(exp(x),1) - 1 + max(x,0): exp on ACT straight from
        # PSUM in parallel with the DVE max; one fused combine after.
        ex = attp.tile([P, OUT], F32, tag=f"ex{t}")
        nc.scalar.activation(ex, ps_o, mybir.ActivationFunctionType.Exp)
        rm1 = attp.tile([P, OUT], F32, tag=f"rm1{t}")
        nc.vector.tensor_scalar(rm1, ps_o, 0.0, -1.0,
                                mybir.AluOpType.max, mybir.AluOpType.add)
        ot = attp.tile([P, OUT], F32, tag=f"ot{t}")
        nc.vector.scalar_tensor_tensor(ot, ex, 1.0, rm1,
                                       mybir.AluOpType.min,
                                       mybir.AluOpType.add)
        oeng = nc.sync if t == 0 else nc.scalar
        oeng.dma_start(out[t * P:(t + 1) * P, :], ot)


def _get_nc():
    global _NC
    if _NC is not None:
        return _NC
    nc = bacc.Bacc("TRN2", target_bir_lowering=False, debug=False,
                   num_devices=NCORES, disable_frame_to_traceback=True)
    hp = nc.dram_tensor("hp", [P, KSUB * M], BF16, kind="ExternalInput").ap()
    wp = nc.dram_tensor("wp", [P, KSUB * OUTE], BF16, kind="ExternalInput").ap()
    eye = nc.dram_tensor("eye", [P, P], F32, kind="ExternalInput").ap()
    masknegT = nc.dram_tensor("masknegT", [P, P], BF16, kind="ExternalInput").ap()
    eyeext = nc.dram_tensor("eyeext", [P, 2 * P], BF16, kind="ExternalInput").ap()
    out = nc.dram_tensor("out", [M, OUT], F32, kind="ExternalOutput").ap()
    with tile.TileContext(nc) as tc:
        with ExitStack() as ctx:
            _build_kernel(ctx, tc, out, hp, wp, eye, masknegT, eyeext)
    nc.compile()
    _NC = nc
    return nc


def kernel(h: np.ndarray, adj: np.ndarray, W: np.ndarray, a: np.ndarray
           ) -> np.ndarray:
    global LAST_EXEC_NS, LAST_RESULTS
    h = np.asarray(h, dtype=np.float32)
    W = np.asarray(W, dtype=np.float32)
    a = np.ascontiguousarray(np.asarray(a, dtype=np.float32)).reshape(2 * OUT)
    assert h.shape == (B, N, IN) and W.shape == (IN, OUT)

    nc = _get_nc()

    # mask [j~, i~]: pair-stacked adj^T on the block diagonal; 0 where
    # attention is allowed, MASKPRE where masked (incl. cross-batch).
    adjT = (np.asarray(adj) > 0).T
    mm = np.zeros((P, P), bool)
    mm[:N, :N] = adjT
    mm[N:, N:] = adjT
    maskneg = np.where(mm, 0.0, MASKPRE).astype(np.float32)
    masknegT = np.ascontiguousarray(maskneg.T).astype(ml_dtypes.bfloat16)
    eye = np.eye(P, dtype=np.float32)
    eyeext = np.concatenate([np.eye(P, dtype=np.float32)] * 2, axis=1)
    eyeext = eyeext.astype(ml_dtypes.bfloat16)

    Wa1 = (W.astype(np.float64) @ a[:OUT].astype(np.float64)).astype(np.float32)
    Wa2 = (W.astype(np.float64) @ a[OUT:].astype(np.float64)).astype(np.float32)
    w_ext = np.concatenate([W, Wa1[:, None], Wa2[:, None]], axis=1)
    w_ext = w_ext.astype(ml_dtypes.bfloat16)
    # repack: wpack[p, c*OUTE + o] = w_ext[c*128 + p, o]
    wpack = np.ascontiguousarray(
        w_ext.reshape(KSUB, P, OUTE).transpose(1, 0, 2).reshape(P, KSUB * OUTE))

    in_maps = []
    for c in range(NCORES):
        hT = h[c * BPC:(c + 1) * BPC].reshape(M, IN).T  # [IN, M]
        hT = hT.astype(ml_dtypes.bfloat16)
        # repack: hpack[p, c*M + m] = hT[c*128 + p, m]
        hpack = np.ascontiguousarray(
            hT.reshape(KSUB, P, M).transpose(1, 0, 2).reshape(P, KSUB * M))
        in_maps.append({
            "hp": hpack, "wp": wpack, "eye": eye,
            "masknegT": masknegT, "eyeext": eyeext,
        })

    trace = os.environ.get("GAT_TRACE", "0") == "1"
    res = run_bass_kernel_spmd(nc, in_maps, list(range(NCORES)), trace=trace)
    LAST_EXEC_NS = res.exec_time_ns
    LAST_RESULTS = res

    out = np.empty((B, N, OUT), np.float32)
    for c in range(NCORES):
        out[c * BPC:(c + 1) * BPC] = res.results[c]["out"].reshape(BPC, N, OUT)
    return out


# revision 7
# speedup vs baseline: 1.0312x; 1.0312x over previous
"""Bass/Tile TRN2 kernel for nn_BatchGraphAttentionLayer (v3).

Reference computation (per batch b):
    Wh  = h[b] @ W                    # [64, 256]
    s1  = Wh @ a[:256], s2 = Wh @ a[256:]
    e   = leaky_relu(s1[i] + s2[j])   # [64, 64] (i rows, j cols)
    att = softmax over axis i of where(adj[i,j]>0, e, -9e15)
    out = elu(att @ Wh)               # h_prime[i] = sum_j att[i,j] Wh[j]

Sharding: data-parallel over batch, 8 cores x 4 batches.

v3 design (from v2, after trace analysis: the kernel is HBM-stream
bound at ~350 GB/s for 16.7 MB/core, with a ~7 us preamble and a
~9 us post-stream tail; the PE ran at 1.2 GHz for the first ~28 us
because HAM saw only intermittent work early):
  - a folded into W on host: w_ext = [W | W@a1 | W@a2] bf16 [16384,258].
  - h and w_ext repacked on host so every DMA slab is one fully
    contiguous per-partition run (hpack/wpack [128, cols]); h streams
    on the sync HWDGE ring, w on the scalar one.  All slab DMAs are
    issued up front into dedicated SBUF buffers (no reuse stalls).
  - ~20 dummy rank-1 matmuls at program start keep the PE busy during
    the DMA-latency window so HAM un-throttles to 2.4 GHz before the
    real stream arrives.
  - attention: batches pair-stacked 2 per 128-partition tile; the
    logit tile for BOTH pairs is built in one PSUM tile [128, 256]:
      e2[j, i+128t] = s1^t[i] + maskpre[j, i]
    via two accumulating matmuls (ones x s1rows, masknegT x [I|I]).
    maskpre = 0 where adj allows, -250 where masked: after the
    leaky-relu (alpha=0.2) that becomes <= -45, so exp() kills the
    entry (rel contribution < 1e-19) -- replaces the explicit 0/1
    mask multiply.
  - softmax: one ACT Prelu (bias = per-partition s2), one ACT Exp
    with accum_out giving the row sums in the same pass, reciprocal +
    scale on DVE, att in bf16.
  - out = elu(att @ Wh) with att/Wh bf16; elu = min(exp(x),1)-1+max(x,0).
"""

import os
from contextlib import ExitStack

import ml_dtypes
import numpy as np

import concourse.bass as bass
import concourse.tile as tile
from concourse import bacc, mybir
from concourse.bass_utils import run_bass_kernel_spmd

F32 = mybir.dt.float32
BF16 = mybir.dt.bfloat16

B, N, IN, OUT = 32, 64, 16384, 256
OUTE = OUT + 2               # w_ext columns (W | Wa1 | Wa2)
NCORES = 8
BPC = B // NCORES            # batches per core = 4
M = BPC * N                  # local rows = 256
P = 128
ALPHA = 0.2
MASKPRE = -250.0             # pre-leaky additive mask (-50 post-leaky)

KSUB = IN // P               # 128 k-subtiles of 128 rows
# taper both ends: fast PE ramp-up, short post-stream crunch.  Middle
# slabs of 8 k-subtiles (1 MB h+w) keep the PE's idle bursts well under
# the ~3.4 us HAM re-throttle window so it stays at 2.4 GHz.
SLABS = [1, 1, 2, 4] + [8] * 14 + [4, 2, 1, 1]
assert sum(SLABS) == KSUB
NWARM = int(os.environ.get("GAT_NWARM", "20"))

_NC = None
LAST_EXEC_NS = None
LAST_RESULTS = None


def _build_kernel(ctx: ExitStack, tc: tile.TileContext, out, hp, wp,
                  eye, masknegT, eyeext):
    nc = tc.nc

    consts = ctx.enter_context(tc.tile_pool(name="consts", bufs=1))
    hpool = ctx.enter_context(tc.tile_pool(name="hslab", bufs=1))
    wpool = ctx.enter_context(tc.tile_pool(name="wslab", bufs=1))
    whpool = ctx.enter_context(tc.tile_pool(name="wh", bufs=1))
    attp = ctx.enter_context(tc.tile_pool(name="att", bufs=1))
    ps_accp = ctx.enter_context(tc.tile_pool(name="psacc", bufs=1, space="PSUM"))
    ps_smallp = ctx.enter_context(tc.tile_pool(name="pssmall", bufs=1, space="PSUM"))
    ps_ep = ctx.enter_context(tc.tile_pool(name="pse", bufs=1, space="PSUM"))
    ps_op = ctx.enter_context(tc.tile_pool(name="pso", bufs=1, space="PSUM"))
    ps_warmp = ctx.enter_context(tc.tile_pool(name="pswarm", bufs=1, space="PSUM"))

    # ---- constants (gpsimd/SWDGE keeps the HWDGE queues clear) ----
    sb_eye = consts.tile([P, P], F32)
    nc.gpsimd.dma_start(sb_eye, eye)
    sb_mneg = consts.tile([P, P], BF16)
    nc.gpsimd.dma_start(sb_mneg, masknegT)
    sb_eyex = consts.tile([P, 2 * P], BF16)
    nc.gpsimd.dma_start(sb_eyex, eyeext)
    # ones row for the rank-1 e-matmul (partition 0 only: legal base)
    sb_ones1 = consts.tile([1, P], BF16)
    nc.vector.memset(sb_ones1, 1.0)
    sb_warm = consts.tile([1, 2 * P], BF16)
    nc.vector.memset(sb_warm, 0.0)

    # ---- stream DMAs: all issued up front, dedicated buffers ----
    hs = []
    ws = []
    k0 = 0
    for s, nsub in enumerate(SLABS):
        ht = hpool.tile([P, nsub * M], BF16, tag=f"h{s}", name=f"h{s}")
        nc.sync.dma_start(ht, hp[:, k0 * M:(k0 + nsub) * M])
        wt = wpool.tile([P, nsub * OUTE], BF16, tag=f"w{s}", name=f"w{s}")
        nc.scalar.dma_start(wt, wp[:, k0 * OUTE:(k0 + nsub) * OUTE])
        hs.append(ht)
        ws.append(wt)
        k0 += nsub

    # ---- HAM warm-up: keep the PE busy while the first slabs land ----
    ps_warm = ps_warmp.tile([P, 2 * P], F32, tag="ps_warm", name="ps_warm")
    for i in range(NWARM):
        nc.tensor.matmul(ps_warm, lhsT=sb_warm[:, 0:P], rhs=sb_warm,
                         start=True, stop=True, skip_group_check=True)

    # ---- projection: Wh_ext accumulated in PSUM (m-major) ----
    ps_wh = [ps_accp.tile([P, OUTE], F32, tag=f"ps_wh{t}", name=f"ps_wh{t}")
             for t in range(2)]
    nslab = len(SLABS)
    k0 = 0
    for s, nsub in enumerate(SLABS):
        first = (s == 0)
        last = (s == nslab - 1)
        for c in range(nsub):
            for t in range(2):
                nc.tensor.matmul(ps_wh[t],
                                 lhsT=hs[s][:, c * M + t * P:c * M + (t + 1) * P],
                                 rhs=ws[s][:, c * OUTE:(c + 1) * OUTE],
                                 start=(first and c == 0),
                                 stop=(last and c == nsub - 1),
                                 skip_group_check=True)
        k0 += nsub

    # ---- logits: e2[j, i+128t] = maskpre[j,i] + s1^t[i] ----
    # mask matmul first: depends only on consts, so the PE can run it
    # in any stream-starvation gap (it sits right after the projection
    # in the FIFO).
    ps_e2 = ps_ep.tile([P, 2 * P], F32, tag="ps_e2", name="ps_e2")
    nc.tensor.matmul(ps_e2, lhsT=sb_mneg, rhs=sb_eyex,
                     start=True, stop=False, skip_group_check=True)

    # scores: s1/s2 columns -> one PE transpose; s2 stays a column and
    # becomes the Prelu bias
    sc = attp.tile([P, 4], F32, tag="sc")
    nc.vector.tensor_copy(out=sc[:, 0:2], in_=ps_wh[0][:, OUT:OUTE])
    nc.scalar.copy(out=sc[:, 2:4], in_=ps_wh[1][:, OUT:OUTE])
    pst0 = ps_smallp.tile([1, P], F32, tag="pst0", name="pst0")
    pst1 = ps_smallp.tile([1, P], F32, tag="pst1", name="pst1")
    nc.tensor.transpose(pst0, sc[:, 0:1], sb_eye)
    nc.tensor.transpose(pst1, sc[:, 2:3], sb_eye)
    s1row = attp.tile([1, 2 * P], BF16, tag="s1row")
    nc.vector.tensor_copy(out=s1row[:, 0:P], in_=pst0[0:1, :])
    nc.scalar.copy(out=s1row[:, P:2 * P], in_=pst1[0:1, :])
    nc.tensor.matmul(ps_e2, lhsT=sb_ones1, rhs=s1row,
                     start=False, stop=True, skip_group_check=True)

    # Wh to SBUF in bf16 (moving operand of the output matmul)
    wh_m = [whpool.tile([P, OUT], BF16, tag=f"wh_m{t}", name=f"wh_m{t}")
            for t in range(2)]
    nc.vector.tensor_copy(out=wh_m[0], in_=ps_wh[0][:, :OUT])
    nc.vector.tensor_copy(out=wh_m[1], in_=ps_wh[1][:, :OUT])

    # ---- softmax: Prelu(+s2), Exp with fused row-sum, scale ----
    rs = attp.tile([P, 2], F32, tag="rs")
    rinv = attp.tile([P, 2], F32, tag="rinv")
    att = attp.tile([P, 2 * P], BF16, tag="attw")
    for t in range(2):
        lk = attp.tile([P, P], F32, tag=f"lk{t}", name=f"lk{t}")
        nc.scalar.activation(lk, ps_e2[:, t * P:(t + 1) * P],
                             mybir.ActivationFunctionType.Prelu,
                             bias=sc[:, 2 * t + 1:2 * t + 2], alpha=ALPHA)
        pexp = attp.tile([P, P], F32, tag=f"pexp{t}", name=f"pexp{t}")
        nc.scalar.activation(pexp, lk, mybir.ActivationFunctionType.Exp,
                             accum_out=rs[:, t:t + 1])
        nc.vector.reciprocal(rinv[:, t:t + 1], rs[:, t:t + 1])
        nc.vector.tensor_scalar_mul(att[:, t * P:(t + 1) * P], pexp,
                                    rinv[:, t:t + 1])

    # ---- out = elu(att @ Wh), pair-interleaved ----
    for t in range(2):
        ps_o = ps_op.tile([P, OUT], F32, tag=f"ps_o{t}", name=f"ps_o{t}")
        nc.tensor.matmul(ps_o, lhsT=att[:, t * P:(t + 1) * P], rhs=wh_m[t],
                         start=True, stop=True, skip_group_check=True)
        # elu(x) = min(exp(x),1) - 1 + max(x,0): exp on ACT straight from
        # PSUM in parallel with the DVE max; one fused combine after.
        ex = attp.tile([P, OUT], F32, tag=f"ex{t}")
        nc.scalar.activation(ex, ps_o, mybir.ActivationFunctionType.Exp)
        rm1 = attp.tile([P, OUT], F32, tag=f"rm1{t}")
        nc.vector.tensor_scalar(rm1, ps_o, 0.0, -1.0,
                                mybir.AluOpType.max, mybir.AluOpType.add)
        ot = attp.tile([P, OUT], BF16, tag=f"ot{t}")
        nc.vector.scalar_tensor_tensor(ot, ex, 1.0, rm1,
                                       mybir.AluOpType.min,
                                       mybir.AluOpType.add)
        oeng = nc.sync if t == 0 else nc.scalar
        oeng.dma_start(out[t * P:(t + 1) * P, :], ot)


def _get_nc():
    global _NC
    if _NC is not None:
        return _NC
    nc = bacc.Bacc("TRN2", target_bir_lowering=False, debug=False,
                   num_devices=NCORES, disable_frame_to_traceback=True)
    hp = nc.dram_tensor("hp", [P, KSUB * M], BF16, kind="ExternalInput").ap()
    wp = nc.dram_tensor("wp", [P, KSUB * OUTE], BF16, kind="ExternalInput").ap()
    eye = nc.dram_tensor("eye", [P, P], F32, kind="ExternalInput").ap()
    masknegT = nc.dram_tensor("masknegT", [P, P], BF16, kind="ExternalInput").ap()
    eyeext = nc.dram_tensor("eyeext", [P, 2 * P], BF16, kind="ExternalInput").ap()
    out = nc.dram_tensor("out", [M, OUT], BF16, kind="ExternalOutput").ap()
    with tile.TileContext(nc) as tc:
        with ExitStack() as ctx:
            _build_kernel(ctx, tc, out, hp, wp, eye, masknegT, eyeext)
    nc.compile()
    _NC = nc
    return nc


def kernel(h: np.ndarray, adj: np.ndarray, W: np.ndarray, a: np.ndarray
           ) -> np.ndarray:
    global LAST_EXEC_NS, LAST_RESULTS
    h = np.asarray(h, dtype=np.float32)
    W = np.asarray(W, dtype=np.float32)
    a = np.ascontiguousarray(np.asarray(a, dtype=np.float32)).reshape(2 * OUT)
    assert h.shape == (B, N, IN) and W.shape == (IN, OUT)

    nc = _get_nc()

    # mask [j~, i~]: pair-stacked adj^T on the block diagonal; 0 where
    # attention is allowed, MASKPRE where masked (incl. cross-batch).
    adjT = (np.asarray(adj) > 0).T
    mm = np.zeros((P, P), bool)
    mm[:N, :N] = adjT
    mm[N:, N:] = adjT
    maskneg = np.where(mm, 0.0, MASKPRE).astype(np.float32)
    masknegT = np.ascontiguousarray(maskneg.T).astype(ml_dtypes.bfloat16)
    eye = np.eye(P, dtype=np.float32)
    eyeext = np.concatenate([np.eye(P, dtype=np.float32)] * 2, axis=1)
    eyeext = eyeext.astype(ml_dtypes.bfloat16)

    Wa1 = (W.astype(np.float64) @ a[:OUT].astype(np.float64)).astype(np.float32)
    Wa2 = (W.astype(np.float64) @ a[OUT:].astype(np.float64)).astype(np.float32)
    w_ext = np.concatenate([W, Wa1[:, None], Wa2[:, None]], axis=1)
    w_ext = w_ext.astype(ml_dtypes.bfloat16)
    # repack: wpack[p, c*OUTE + o] = w_ext[c*128 + p, o]
    wpack = np.ascontiguousarray(
        w_ext.reshape(KSUB, P, OUTE).transpose(1, 0, 2).reshape(P, KSUB * OUTE))

    in_maps = []
    for c in range(NCORES):
        hT = h[c * BPC:(c + 1) * BPC].reshape(M, IN).T  # [IN, M]
        hT = hT.astype(ml_dtypes.bfloat16)
        # repack: hpack[p, c*M + m] = hT[c*128 + p, m]
        hpack = np.ascontiguousarray(
            hT.reshape(KSUB, P, M).transpose(1, 0, 2).reshape(P, KSUB * M))
        in_maps.append({
            "hp": hpack, "wp": wpack, "eye": eye,
            "masknegT": masknegT, "eyeext": eyeext,
        })

    trace = os.environ.get("GAT_TRACE", "0") == "1"
    res = run_bass_kernel_spmd(nc, in_maps, list(range(NCORES)), trace=trace)
    LAST_EXEC_NS = res.exec_time_ns
    LAST_RESULTS = res

    out = np.empty((B, N, OUT), np.float32)
    for c in range(NCORES):
        out[c * BPC:(c + 1) * BPC] = np.asarray(
            res.results[c]["out"], dtype=np.float32).reshape(BPC, N, OUT)
    return out
